# revision 1
# baseline (speedup 1.0000x reference)
"""Trainium2 Bass kernel for nn_PostAttention (sparse_attention) — v8.

Computation (B=1, N=4096, H=8, d_qk=96, d_v=64):
    proj = qk @ W_qk -> q, k per head;  v = v_cls @ W_v per head
    S = q @ k.T * scale;  E = exp(S);  Z_i = sum_j E
    out_i = sum_j E_ij * m_ij * v_j / (Z_i * H * M_i),  M_i = sum_j m_ij

Sharding: 1 head per core (8 cores, 8 heads); each core handles all 4096
queries for its head.  Everything computed transposed (S^T = [key j on
partitions, query i on free]) so exp/mask outputs feed the P@V matmul as
the moving operand with no transpose of the attention matrix.  All data
fp16 (fp8 fails: the P@V sum is sign-incoherent so quantization noise
does not average out; measured 5-7% error from fp8 P/V/mask).

Projection phase (DMA-bound, ~35us): chunk inputs split across BOTH
hwdge queues (qk on SP, v on ScalarE); V j-tiles transposed on the PE
inline per chunk.  Attention (9 ACT groups of 4/3 j-tiles per i-chunk):
  PE      : S matmuls, PV accumulation, Z ones-matmuls for groups 1/3/5
            + 4 Z-finalize matmuls over the DVE accumulator
  ScalarE : exp batched per group across 7 PSUM banks + finalize copies
  DVE     : everything else — mask-muls + Z accumulation for the other
            groups (GpSimd is NOT used: concurrent GpSimd tensor ops
            were measured to slow DVE 3.3x via SBUF contention)
All DVE tensor ops use FLAT 2-D APs; in-place DVE ops are avoided (both
measured pitfalls).  The o bank alternates halves per ic
(tile_position) so consecutive ics don't serialize; the Z strip is the
other half's first row (DVE-zeroed, all Z matmuls start=False; the PV
start=True at jt0 is the bank's only has_written clear).  S(ic+1, g0)
is emitted before ic's PV tail + Z finalize to kill the ic-boundary
bubble.  M_i and the final 1/(Z*H*M) scaling happen on the host; Z is
exported per core as a [1, N] f32 row.
"""
import os
import sys

sys.path.insert(0, "/opt/trn_rl_repo")
import numpy as np

import concourse.bass as bass
import concourse.mybir as mybir
import concourse.tile as tile
from concourse import bacc
from concourse.bass_utils import run_bass_kernel_spmd
from concourse.masks import make_identity

f32 = mybir.dt.float32
f16 = mybir.dt.float16
FT = mybir.ActivationFunctionType

N = 4096
H = 8
DQK = 96
DV = 64
NIC = 8            # i-chunks of 512 queries
NJT = 32           # j-tiles of 128 keys
SCALE = (256 // 8) ** -0.5
EXP_BIAS = -4.0    # uniform shift inside exp; cancels in the Z ratio

# group order ends each ic with a 3-group so the s4 ring never has
# back-to-back uses across the ic boundary (kills a ~6us bubble + HAM
# re-throttle per ic); the unavoidable 4,4 adjacency sits at g6/g7 where
# the PE has PV backlog to chew during the wait
GROUPS = [(0, 4), (4, 3), (7, 4), (11, 3), (14, 4), (18, 3), (21, 4), (25, 4), (29, 3)]
PE_Z_GROUPS = {1, 3, 5}   # Z via per-tile ones-matmuls on the PE

_CACHED = {}


def _build_nc():
    nc = bacc.Bacc(name="post_attention_v8")

    qkT = nc.declare_dram_parameter("qkT", [768, N], f16, isOutput=False)
    vT = nc.declare_dram_parameter("vT", [512, N], f16, isOutput=False)
    wq = nc.declare_dram_parameter("wq", [768, DQK], f16, isOutput=False)
    wk = nc.declare_dram_parameter("wk", [768, DQK], f16, isOutput=False)
    wv = nc.declare_dram_parameter("wv", [512, DV], f16, isOutput=False)
    maskT = nc.declare_dram_parameter("maskT", [N, N], f16, isOutput=False)
    outT = nc.declare_dram_parameter("outT", [DV, N], f32, isOutput=True)
    zout = nc.declare_dram_parameter("zout", [1, N], f32, isOutput=True)

    with tile.TileContext(nc) as tc:
        with (
            tc.tile_pool(name="const", bufs=1) as const,
            tc.tile_pool(name="persist", bufs=1) as persist,
        ):
            ones16 = const.tile([128, 1], f16)
            nc.vector.memset(ones16, 1.0)
            bias_t = const.tile([128, 1], f32)
            nc.vector.memset(bias_t, EXP_BIAS)
            ident16 = const.tile([128, 128], f16)
            make_identity(nc, ident16)

            QT = persist.tile([DQK, N], f16)
            KT = persist.tile([DQK, N], f16)
            V = persist.tile([128, NJT, DV], f16)

            # ---------------- projection phase ----------------
            with (
                tc.tile_pool(name="wpool", bufs=1) as wpool,
                tc.tile_pool(name="vt16p", bufs=1) as vt16p,
                tc.tile_pool(name="qs", bufs=3) as qs,
                tc.tile_pool(name="pp", bufs=2, space="PSUM") as pp,
                tc.tile_pool(name="ptr", bufs=2, space="PSUM") as ptr,
            ):
                # HAM warm-up: keep the PE busy during the initial DMA wait
                # so the clock gate opens before the first projection matmuls
                warm_ps = pp.tile([DQK, 512], f32, tag="kt", name="warm")
                for i in range(80):
                    nc.tensor.matmul(
                        warm_ps[:, 0:64], lhsT=ident16[:, 0:DQK],
                        rhs=ident16[:, 0:64],
                        start=True, stop=True, skip_group_check=True,
                    )
                wq_t = wpool.tile([128, 6, DQK], f16)
                nc.sync.dma_start(out=wq_t, in_=wq.rearrange("(t p) m -> p t m", p=128))
                wk_t = wpool.tile([128, 6, DQK], f16)
                nc.sync.dma_start(out=wk_t, in_=wk.rearrange("(t p) m -> p t m", p=128))
                wv_t = wpool.tile([128, 4, DV], f16)
                nc.scalar.dma_start(out=wv_t, in_=wv.rearrange("(t p) m -> p t m", p=128))
                VT16 = vt16p.tile([DV, N], f16)

                for n in range(8):
                    ncol = slice(n * 512, (n + 1) * 512)
                    qk_sl = qs.tile([128, 6, 512], f16, tag="qksl")
                    nc.sync.dma_start(
                        out=qk_sl, in_=qkT[:, ncol].rearrange("(t p) n -> p t n", p=128)
                    )
                    v_sl = qs.tile([128, 4, 512], f16, tag="vsl")
                    nc.scalar.dma_start(
                        out=v_sl, in_=vT[:, ncol].rearrange("(t p) n -> p t n", p=128)
                    )

                    kt_ps = pp.tile([DQK, 512], f32, tag="kt")
                    for c in range(6):
                        nc.tensor.matmul(
                            kt_ps, lhsT=wk_t[:, c, :], rhs=qk_sl[:, c, :],
                            start=(c == 0), stop=(c == 5),
                        )
                    nc.scalar.copy(KT[:, ncol], kt_ps)

                    qt_ps = pp.tile([DQK, 512], f32, tag="qt")
                    for c in range(6):
                        nc.tensor.matmul(
                            qt_ps, lhsT=wq_t[:, c, :], rhs=qk_sl[:, c, :],
                            start=(c == 0), stop=(c == 5),
                        )
                    nc.scalar.copy(QT[:, ncol], qt_ps)

                    vt_ps = pp.tile([DV, 512], f32, tag="vt")
                    for c in range(4):
                        nc.tensor.matmul(
                            vt_ps, lhsT=wv_t[:, c, :], rhs=v_sl[:, c, :],
                            start=(c == 0), stop=(c == 3),
                        )
                    nc.scalar.copy(VT16[:, ncol], vt_ps)

                    # transpose the PREVIOUS chunk's V j-tiles (one-chunk
                    # lag keeps the transposes off the VT16-copy wait)
                    for m in ([n - 1] if n >= 1 else []) + ([n] if n == 7 else []):
                        tr = ptr.tile([128, 4, DV], f16, tag="tr", name=f"tr{m}")
                        for a in range(4):
                            jt = 4 * m + a
                            nc.tensor.transpose(
                                tr[:, a, :],
                                VT16[:, jt * 128 : (jt + 1) * 128],
                                ident16[0:DV, 0:DV],
                            )
                        nc.vector.tensor_copy(
                            V[:, 4 * m : 4 * m + 4, :].rearrange("p a n -> p (a n)"),
                            tr.rearrange("p a n -> p (a n)"),
                        )

            # ---------------- attention phase ----------------
            with (
                tc.tile_pool(name="mt", bufs=3) as mtp,
                tc.tile_pool(name="ep", bufs=2) as ep,
                tc.tile_pool(name="p16", bufs=2) as p16,
                tc.tile_pool(name="zp", bufs=2) as zp,
                tc.tile_pool(name="fin", bufs=2) as fin,
                tc.tile_pool(name="sp4", bufs=1, space="PSUM") as sp4,
                tc.tile_pool(name="sp3", bufs=1, space="PSUM") as sp3,
                tc.tile_pool(name="op", bufs=1, space="PSUM") as op,
            ):
                o_bank = op.tile([128, 512], f32)
                state = {}

                def start_ic(ic):
                    state[ic] = {
                        "ep": ep.tile([128, NJT, 512], f16, tag="e", name=f"ep{ic}"),
                        "p": p16.tile([128, NJT, 512], f16, tag="p", name=f"p{ic}"),
                        "zacc": zp.tile([128, 4, 512], f16, tag="z", name=f"za{ic}"),
                        "next_tile": 0,
                        "zn": 0,
                    }

                def group_of(jt):
                    for g, (g0, gsz) in enumerate(GROUPS):
                        if g0 <= jt < g0 + gsz:
                            return g
                    return None

                def emit_pv(ic, limit):
                    st = state[ic]
                    o_lo = o_bank[0:DV, :]
                    z_row = o_bank[64:65, :]
                    while st["next_tile"] < NJT and st["next_tile"] < limit:
                        jt = st["next_tile"]
                        nc.tensor.matmul(
                            o_lo,
                            lhsT=V[:, jt, :],
                            rhs=st["p"][:, jt, :],
                            start=(jt == 0), stop=(jt == NJT - 1),
                            skip_group_check=True,
                        )
                        if group_of(jt) in PE_Z_GROUPS:
                            nc.tensor.matmul(
                                z_row, lhsT=ones16, rhs=st["ep"][:, jt, :],
                                start=False, stop=False,
                                tile_position=(0, 64), skip_group_check=True,
                            )
                        st["next_tile"] += 1

                def emit_group_S(ic, g):
                    g0, gsz = GROUPS[g]
                    icol = slice(ic * 512, (ic + 1) * 512)
                    jr = slice(g0 * 128, (g0 + gsz) * 128)
                    m_g = mtp.tile([128, gsz, 512], f16, tag=f"m{gsz}")
                    nc.sync.dma_start(
                        out=m_g, in_=maskT[jr, icol].rearrange("(a p) n -> p a n", p=128)
                    )
                    pool = sp4 if gsz == 4 else sp3
                    s_t = pool.tile([128, gsz, 512], f32, tag=f"s{gsz}")
                    for a in range(gsz):
                        jt = g0 + a
                        nc.tensor.matmul(
                            s_t[:, a, :],
                            lhsT=KT[:, jt * 128 : (jt + 1) * 128],
                            rhs=QT[:, icol],
                            start=True, stop=True,
                        )
                    if g >= 1:
                        emit_pv(ic, GROUPS[g - 1][0])
                    return s_t, m_g

                def emit_group_consumers(ic, g, s_t, m_g):
                    g0, gsz = GROUPS[g]
                    st = state[ic]
                    gsl = slice(g0, g0 + gsz)
                    nc.scalar.activation(
                        st["ep"][:, gsl, :], s_t, FT.Exp, bias=bias_t, scale=SCALE
                    )
                    e_fl = st["ep"][:, gsl, :].rearrange("p a n -> p (a n)")
                    m_fl = m_g.rearrange("p a n -> p (a n)")
                    if g not in PE_Z_GROUPS:
                        z_fl = st["zacc"][:, 0:gsz, :].rearrange("p a n -> p (a n)")
                        if st["zn"] == 0:
                            nc.vector.tensor_copy(z_fl, e_fl)
                        else:
                            nc.vector.tensor_add(z_fl, z_fl, e_fl)
                        st["zn"] += 1
                    p_fl = st["p"][:, gsl, :].rearrange("p a n -> p (a n)")
                    nc.vector.tensor_mul(p_fl, e_fl, m_fl)

                def finish_ic(ic):
                    st = state[ic]
                    emit_pv(ic, NJT)
                    z_row = o_bank[64:65, :]
                    for k in range(4):
                        nc.tensor.matmul(
                            z_row, lhsT=ones16, rhs=st["zacc"][:, k, :],
                            start=False, stop=(k == 3),
                            tile_position=(0, 64), skip_group_check=True,
                        )

                def flush_fin(ic):
                    st = state[ic]
                    icol = slice(ic * 512, (ic + 1) * 512)
                    out_sb = fin.tile([DV, 512], f32, tag="o")
                    nc.scalar.copy(out_sb, o_bank[0:DV, :])
                    z_sb = fin.tile([1, 512], f32, tag="z")
                    nc.scalar.copy(z_sb, o_bank[64:65, :])
                    nc.sync.dma_start(out=zout[0:1, icol], in_=z_sb)
                    nc.sync.dma_start(out=outT[:, icol], in_=out_sb)
                    del state[ic]

                start_ic(0)
                nc.vector.memset(o_bank[64:65, :], 0.0)
                for g in range(9):
                    s_t, m_g = emit_group_S(0, g)
                    emit_group_consumers(0, g, s_t, m_g)

                for ic in range(1, NIC):
                    start_ic(ic)
                    for g in range(9):
                        s_t, m_g = emit_group_S(ic, g)
                        if g == 0:
                            finish_ic(ic - 1)
                        emit_group_consumers(ic, g, s_t, m_g)
                        if g == 1:
                            # must precede PV(ic, jt0) (emitted at g2), which
                            # overwrites o_bank[0:64] and clears the bank's
                            # has_written bits
                            flush_fin(ic - 1)
                            nc.vector.memset(o_bank[64:65, :], 0.0)
                finish_ic(NIC - 1)
                flush_fin(NIC - 1)

    nc.finalize()
    return nc


def kernel(**inputs) -> np.ndarray:
    qk = np.asarray(inputs["qk"], dtype=np.float32)        # [1, N, 768]
    v_cls = np.asarray(inputs["v_cls"], dtype=np.float32)  # [1, N, 512]
    masks = np.asarray(inputs["masks"], dtype=np.float32)  # [1, N, N]
    W_qk = np.asarray(inputs["W_qk"], dtype=np.float32)    # [768, 1536]
    W_v = np.asarray(inputs["W_v"], dtype=np.float32)      # [512, 512]

    if "nc" not in _CACHED:
        _CACHED["nc"] = _build_nc()
    nc = _CACHED["nc"]

    qkT_h = np.ascontiguousarray(qk[0].T).astype(np.float16)
    vT_h = np.ascontiguousarray(v_cls[0].T).astype(np.float16)
    maskT_h = np.ascontiguousarray(masks[0].T).astype(np.float16)
    M = masks[0].astype(np.float64).sum(axis=1)            # [N] row sums

    in_maps = []
    for h in range(8):
        in_maps.append({
            "qkT": qkT_h,
            "vT": vT_h,
            "wq": np.ascontiguousarray(W_qk[:, h * DQK : (h + 1) * DQK]).astype(np.float16),
            "wk": np.ascontiguousarray(W_qk[:, 768 + h * DQK : 768 + (h + 1) * DQK]).astype(np.float16),
            "wv": np.ascontiguousarray(W_v[:, h * DV : (h + 1) * DV]).astype(np.float16),
            "maskT": maskT_h,
        })

    trace = os.environ.get("KERNEL_TRACE", "0") == "1"
    res = run_bass_kernel_spmd(nc, in_maps, list(range(8)), trace=trace)
    if trace:
        _CACHED["exec_time_ns"] = res.exec_time_ns
        _CACHED["mean_exec_time_ns"] = res.mean_exec_time_ns

    out = np.empty((1, N, 512), dtype=np.float32)
    for h in range(8):
        oT = res.results[h]["outT"].astype(np.float64)     # [64, N]
        z = res.results[h]["zout"][0].astype(np.float64)   # [N]
        w = 1.0 / (H * M * z)
        out[0, :, h * DV : (h + 1) * DV] = (oT * w[None, :]).T.astype(np.float32)
    return out



# revision 3
# speedup vs baseline: 1.0156x; 1.0156x over previous
"""Trainium2 Bass kernel for nn_PostAttention (sparse_attention) — v8.

Computation (B=1, N=4096, H=8, d_qk=96, d_v=64):
    proj = qk @ W_qk -> q, k per head;  v = v_cls @ W_v per head
    S = q @ k.T * scale;  E = exp(S);  Z_i = sum_j E
    out_i = sum_j E_ij * m_ij * v_j / (Z_i * H * M_i),  M_i = sum_j m_ij

Sharding: 1 head per core (8 cores, 8 heads); each core handles all 4096
queries for its head.  Everything computed transposed (S^T = [key j on
partitions, query i on free]) so exp/mask outputs feed the P@V matmul as
the moving operand with no transpose of the attention matrix.  All data
fp16 (fp8 fails: the P@V sum is sign-incoherent so quantization noise
does not average out; measured 5-7% error from fp8 P/V/mask).

Projection phase (DMA-bound, ~35us): chunk inputs split across BOTH
hwdge queues (qk on SP, v on ScalarE); V j-tiles transposed on the PE
inline per chunk.  Attention (9 ACT groups of 4/3 j-tiles per i-chunk):
  PE      : S matmuls, PV accumulation, Z ones-matmuls for groups 1/3/5
            + 4 Z-finalize matmuls over the DVE accumulator
  ScalarE : exp batched per group across 7 PSUM banks + finalize copies
  DVE     : everything else — mask-muls + Z accumulation for the other
            groups (GpSimd is NOT used: concurrent GpSimd tensor ops
            were measured to slow DVE 3.3x via SBUF contention)
All DVE tensor ops use FLAT 2-D APs; in-place DVE ops are avoided (both
measured pitfalls).  The o bank alternates halves per ic
(tile_position) so consecutive ics don't serialize; the Z strip is the
other half's first row (DVE-zeroed, all Z matmuls start=False; the PV
start=True at jt0 is the bank's only has_written clear).  S(ic+1, g0)
is emitted before ic's PV tail + Z finalize to kill the ic-boundary
bubble.  M_i and the final 1/(Z*H*M) scaling happen on the host; Z is
exported per core as a [1, N] f32 row.
"""
import os
import sys

sys.path.insert(0, "/opt/trn_rl_repo")
import numpy as np

import concourse.bass as bass
import concourse.mybir as mybir
import concourse.tile as tile
from concourse import bacc
from concourse.bass_utils import run_bass_kernel_spmd
from concourse.masks import make_identity

f32 = mybir.dt.float32
f16 = mybir.dt.float16
FT = mybir.ActivationFunctionType

N = 4096
H = 8
DQK = 96
DV = 64
NIC = 8            # i-chunks of 512 queries
NJT = 32           # j-tiles of 128 keys
SCALE = (256 // 8) ** -0.5
EXP_BIAS = -4.0    # uniform shift inside exp; cancels in the Z ratio

# group order ends each ic with a 3-group so the s4 ring never has
# back-to-back uses across the ic boundary (kills a ~6us bubble + HAM
# re-throttle per ic); the unavoidable 4,4 adjacency sits at g6/g7 where
# the PE has PV backlog to chew during the wait
GROUPS = [(0, 4), (4, 3), (7, 4), (11, 3), (14, 4), (18, 3), (21, 4), (25, 4), (29, 3)]
PE_Z_GROUPS = set()       # all Z accumulation on the DVE (frees ~40us of PE)

_CACHED = {}


def _build_nc():
    nc = bacc.Bacc(name="post_attention_v8")

    qkT = nc.declare_dram_parameter("qkT", [768, N], f16, isOutput=False)
    vT = nc.declare_dram_parameter("vT", [512, N], f16, isOutput=False)
    wq = nc.declare_dram_parameter("wq", [768, DQK], f16, isOutput=False)
    wk = nc.declare_dram_parameter("wk", [768, DQK], f16, isOutput=False)
    wv = nc.declare_dram_parameter("wv", [512, DV], f16, isOutput=False)
    maskT = nc.declare_dram_parameter("maskT", [N, N], f16, isOutput=False)
    outT = nc.declare_dram_parameter("outT", [DV, N], f32, isOutput=True)
    zout = nc.declare_dram_parameter("zout", [1, N], f32, isOutput=True)

    with tile.TileContext(nc) as tc:
        with (
            tc.tile_pool(name="const", bufs=1) as const,
            tc.tile_pool(name="persist", bufs=1) as persist,
        ):
            ones16 = const.tile([128, 1], f16)
            nc.vector.memset(ones16, 1.0)
            bias_t = const.tile([128, 1], f32)
            nc.vector.memset(bias_t, EXP_BIAS)
            ident16 = const.tile([128, 128], f16)
            make_identity(nc, ident16)

            QT = persist.tile([DQK, N], f16)
            KT = persist.tile([DQK, N], f16)
            V = persist.tile([128, NJT, DV], f16)

            # ---------------- projection phase ----------------
            with (
                tc.tile_pool(name="wpool", bufs=1) as wpool,
                tc.tile_pool(name="vt16p", bufs=1) as vt16p,
                tc.tile_pool(name="qs", bufs=3) as qs,
                tc.tile_pool(name="pp", bufs=2, space="PSUM") as pp,
                tc.tile_pool(name="ptr", bufs=2, space="PSUM") as ptr,
            ):
                # HAM warm-up: keep the PE busy during the initial DMA wait
                # so the clock gate opens before the first projection matmuls
                warm_ps = pp.tile([DQK, 512], f32, tag="kt", name="warm")
                for i in range(80):
                    nc.tensor.matmul(
                        warm_ps[:, 0:64], lhsT=ident16[:, 0:DQK],
                        rhs=ident16[:, 0:64],
                        start=True, stop=True, skip_group_check=True,
                    )
                wq_t = wpool.tile([128, 6, DQK], f16)
                nc.sync.dma_start(out=wq_t, in_=wq.rearrange("(t p) m -> p t m", p=128))
                wk_t = wpool.tile([128, 6, DQK], f16)
                nc.sync.dma_start(out=wk_t, in_=wk.rearrange("(t p) m -> p t m", p=128))
                wv_t = wpool.tile([128, 4, DV], f16)
                nc.scalar.dma_start(out=wv_t, in_=wv.rearrange("(t p) m -> p t m", p=128))
                VT16 = vt16p.tile([DV, N], f16)

                for n in range(8):
                    ncol = slice(n * 512, (n + 1) * 512)
                    qk_sl = qs.tile([128, 6, 512], f16, tag="qksl")
                    nc.sync.dma_start(
                        out=qk_sl, in_=qkT[:, ncol].rearrange("(t p) n -> p t n", p=128)
                    )
                    v_sl = qs.tile([128, 4, 512], f16, tag="vsl")
                    nc.scalar.dma_start(
                        out=v_sl, in_=vT[:, ncol].rearrange("(t p) n -> p t n", p=128)
                    )

                    kt_ps = pp.tile([DQK, 512], f32, tag="kt")
                    for c in range(6):
                        nc.tensor.matmul(
                            kt_ps, lhsT=wk_t[:, c, :], rhs=qk_sl[:, c, :],
                            start=(c == 0), stop=(c == 5),
                        )
                    nc.scalar.copy(KT[:, ncol], kt_ps)

                    qt_ps = pp.tile([DQK, 512], f32, tag="qt")
                    for c in range(6):
                        nc.tensor.matmul(
                            qt_ps, lhsT=wq_t[:, c, :], rhs=qk_sl[:, c, :],
                            start=(c == 0), stop=(c == 5),
                        )
                    nc.scalar.copy(QT[:, ncol], qt_ps)

                    vt_ps = pp.tile([DV, 512], f32, tag="vt")
                    for c in range(4):
                        nc.tensor.matmul(
                            vt_ps, lhsT=wv_t[:, c, :], rhs=v_sl[:, c, :],
                            start=(c == 0), stop=(c == 3),
                        )
                    nc.scalar.copy(VT16[:, ncol], vt_ps)

                    # transpose the PREVIOUS chunk's V j-tiles (one-chunk
                    # lag keeps the transposes off the VT16-copy wait)
                    for m in ([n - 1] if n >= 1 else []) + ([n] if n == 7 else []):
                        tr = ptr.tile([128, 4, DV], f16, tag="tr", name=f"tr{m}")
                        for a in range(4):
                            jt = 4 * m + a
                            nc.tensor.transpose(
                                tr[:, a, :],
                                VT16[:, jt * 128 : (jt + 1) * 128],
                                ident16[0:DV, 0:DV],
                            )
                        nc.vector.tensor_copy(
                            V[:, 4 * m : 4 * m + 4, :].rearrange("p a n -> p (a n)"),
                            tr.rearrange("p a n -> p (a n)"),
                        )

            # ---------------- attention phase ----------------
            with (
                tc.tile_pool(name="mt", bufs=3) as mtp,
                tc.tile_pool(name="ep", bufs=2) as ep,
                tc.tile_pool(name="p16", bufs=2) as p16,
                tc.tile_pool(name="zp", bufs=2) as zp,
                tc.tile_pool(name="fin", bufs=2) as fin,
                tc.tile_pool(name="sp4", bufs=1, space="PSUM") as sp4,
                tc.tile_pool(name="sp3", bufs=1, space="PSUM") as sp3,
                tc.tile_pool(name="op", bufs=1, space="PSUM") as op,
            ):
                o_bank = op.tile([128, 512], f32)
                state = {}

                def start_ic(ic):
                    state[ic] = {
                        "ep": ep.tile([128, NJT, 512], f16, tag="e", name=f"ep{ic}"),
                        "p": p16.tile([128, NJT, 512], f16, tag="p", name=f"p{ic}"),
                        "zacc": zp.tile([128, 4, 512], f16, tag="z", name=f"za{ic}"),
                        "next_tile": 0,
                        "zn": 0,
                    }

                def emit_pv(ic, limit):
                    st = state[ic]
                    o_lo = o_bank[0:DV, :]
                    while st["next_tile"] < NJT and st["next_tile"] < limit:
                        jt = st["next_tile"]
                        nc.tensor.matmul(
                            o_lo,
                            lhsT=V[:, jt, :],
                            rhs=st["p"][:, jt, :],
                            start=(jt == 0), stop=(jt == NJT - 1),
                            skip_group_check=True,
                        )
                        st["next_tile"] += 1

                def emit_group_S(ic, g):
                    g0, gsz = GROUPS[g]
                    icol = slice(ic * 512, (ic + 1) * 512)
                    jr = slice(g0 * 128, (g0 + gsz) * 128)
                    m_g = mtp.tile([128, gsz, 512], f16, tag=f"m{gsz}")
                    nc.sync.dma_start(
                        out=m_g, in_=maskT[jr, icol].rearrange("(a p) n -> p a n", p=128)
                    )
                    pool = sp4 if gsz == 4 else sp3
                    s_t = pool.tile([128, gsz, 512], f32, tag=f"s{gsz}")
                    for a in range(gsz):
                        jt = g0 + a
                        nc.tensor.matmul(
                            s_t[:, a, :],
                            lhsT=KT[:, jt * 128 : (jt + 1) * 128],
                            rhs=QT[:, icol],
                            start=True, stop=True,
                        )
                    if g >= 1:
                        emit_pv(ic, GROUPS[g - 1][0])
                    return s_t, m_g

                def emit_group_consumers(ic, g, s_t, m_g):
                    g0, gsz = GROUPS[g]
                    st = state[ic]
                    gsl = slice(g0, g0 + gsz)
                    nc.scalar.activation(
                        st["ep"][:, gsl, :], s_t, FT.Exp, bias=bias_t, scale=SCALE
                    )
                    e_fl = st["ep"][:, gsl, :].rearrange("p a n -> p (a n)")
                    m_fl = m_g.rearrange("p a n -> p (a n)")
                    if g not in PE_Z_GROUPS:
                        z_fl = st["zacc"][:, 0:gsz, :].rearrange("p a n -> p (a n)")
                        if st["zn"] == 0:
                            nc.vector.tensor_copy(z_fl, e_fl)
                        else:
                            nc.vector.tensor_add(z_fl, z_fl, e_fl)
                        st["zn"] += 1
                    p_fl = st["p"][:, gsl, :].rearrange("p a n -> p (a n)")
                    nc.vector.tensor_mul(p_fl, e_fl, m_fl)

                def finish_ic(ic):
                    st = state[ic]
                    emit_pv(ic, NJT)
                    z_row = o_bank[64:65, :]
                    for k in range(4):
                        nc.tensor.matmul(
                            z_row, lhsT=ones16, rhs=st["zacc"][:, k, :],
                            start=False, stop=(k == 3),
                            tile_position=(0, 64), skip_group_check=True,
                        )

                def flush_fin(ic):
                    st = state[ic]
                    icol = slice(ic * 512, (ic + 1) * 512)
                    out_sb = fin.tile([DV, 512], f32, tag="o")
                    nc.scalar.copy(out_sb, o_bank[0:DV, :])
                    z_sb = fin.tile([1, 512], f32, tag="z")
                    nc.scalar.copy(z_sb, o_bank[64:65, :])
                    nc.sync.dma_start(out=zout[0:1, icol], in_=z_sb)
                    nc.sync.dma_start(out=outT[:, icol], in_=out_sb)
                    del state[ic]

                start_ic(0)
                nc.vector.memset(o_bank[64:65, :], 0.0)
                for g in range(9):
                    s_t, m_g = emit_group_S(0, g)
                    emit_group_consumers(0, g, s_t, m_g)

                for ic in range(1, NIC):
                    start_ic(ic)
                    for g in range(9):
                        s_t, m_g = emit_group_S(ic, g)
                        if g == 0:
                            finish_ic(ic - 1)
                        emit_group_consumers(ic, g, s_t, m_g)
                        if g == 1:
                            # must precede PV(ic, jt0) (emitted at g2), which
                            # overwrites o_bank[0:64] and clears the bank's
                            # has_written bits
                            flush_fin(ic - 1)
                            nc.vector.memset(o_bank[64:65, :], 0.0)
                finish_ic(NIC - 1)
                flush_fin(NIC - 1)

    nc.finalize()
    return nc


def kernel(**inputs) -> np.ndarray:
    qk = np.asarray(inputs["qk"], dtype=np.float32)        # [1, N, 768]
    v_cls = np.asarray(inputs["v_cls"], dtype=np.float32)  # [1, N, 512]
    masks = np.asarray(inputs["masks"], dtype=np.float32)  # [1, N, N]
    W_qk = np.asarray(inputs["W_qk"], dtype=np.float32)    # [768, 1536]
    W_v = np.asarray(inputs["W_v"], dtype=np.float32)      # [512, 512]

    if "nc" not in _CACHED:
        _CACHED["nc"] = _build_nc()
    nc = _CACHED["nc"]

    qkT_h = np.ascontiguousarray(qk[0].T).astype(np.float16)
    vT_h = np.ascontiguousarray(v_cls[0].T).astype(np.float16)
    maskT_h = np.ascontiguousarray(masks[0].T).astype(np.float16)
    M = masks[0].astype(np.float64).sum(axis=1)            # [N] row sums

    in_maps = []
    for h in range(8):
        in_maps.append({
            "qkT": qkT_h,
            "vT": vT_h,
            "wq": np.ascontiguousarray(W_qk[:, h * DQK : (h + 1) * DQK]).astype(np.float16),
            "wk": np.ascontiguousarray(W_qk[:, 768 + h * DQK : 768 + (h + 1) * DQK]).astype(np.float16),
            "wv": np.ascontiguousarray(W_v[:, h * DV : (h + 1) * DV]).astype(np.float16),
            "maskT": maskT_h,
        })

    trace = os.environ.get("KERNEL_TRACE", "0") == "1"
    res = run_bass_kernel_spmd(nc, in_maps, list(range(8)), trace=trace)
    if trace:
        _CACHED["exec_time_ns"] = res.exec_time_ns
        _CACHED["mean_exec_time_ns"] = res.mean_exec_time_ns

    out = np.empty((1, N, 512), dtype=np.float32)
    for h in range(8):
        oT = res.results[h]["outT"].astype(np.float64)     # [64, N]
        z = res.results[h]["zout"][0].astype(np.float64)   # [N]
        w = 1.0 / (H * M * z)
        out[0, :, h * DV : (h + 1) * DV] = (oT * w[None, :]).T.astype(np.float32)
    return out

